# revision 8
# baseline (speedup 1.0000x reference)
"""DSimilarity.gradgrad force-force covariance block on 8 Trainium2 cores.

out[m*3+a, n*3+b] = sum_{i,j} u1[i,a]*u2[j,b]*gg[i,j]*[i1[i]==m]*[i2[j]==n]
with gg[i,j] = (c - c^2 diff^2) * exp(-0.5 c diff^2), diff = d1[i]-d2[j], c=1/l^2.

gg depends only on the scalar difference d1[i]-d2[j], so the 4000x4000 kernel
matrix separates: a 2D Chebyshev expansion of gg on the observed d-range,
truncated by SVD, gives gg ~= sum_k phi_k(d1) psi_k(d2) with rank ~16 at
machine precision (rank 32 used for margin). Folding the scatter matrices in
on the host reduces the whole computation to out = M^T @ W with
M[k, 3m+a] = sum_{i1[i]=m} phi_k(d1_i) u1[i,a]   (shared by all cores)
W[k, 3n+b] = sum_{i2[j]=n} psi_k(d2_j) u2[j,b]   (column strip per core).

Each core computes a 62-63-atom strip of output columns, transposed
(out columns on partitions, rows on the free axis) so the whole strip is
4 bf16 matmuls: 2 stationary W-chunks x 2 moving M-chunks (1024+512).
All IO is bf16 (~0.7MB/core); out rides both HWDGE rings.
"""

import math
import sys
import types

import numpy as np

NCORES = 8
CHEB_K = 64  # chebyshev grid size for the 2D expansion
R = 32       # separation rank (machine precision by ~24)

TRACE = False  # test.py sets True to capture an NTFF profile
LAST_RESULTS = None  # BassKernelResults of the last run (for test.py)

_PROGRAM_CACHE = {}


def _install_ntff_hook():
    try:
        from antenv.axon_hooks import get_axon_ntff_profile_hook  # noqa: F401
        return
    except ImportError:
        pass
    try:
        from trn_agent_boot.trn_boot import _ntff_profile_via_ctypes
        import antenv
        hook = _ntff_profile_via_ctypes('/opt/axon/libaxon_pjrt.so')
        mod = types.ModuleType("antenv.axon_hooks")
        mod._hook = hook
        mod.get_axon_ntff_profile_hook = lambda: mod._hook
        mod.set_axon_ntff_profile_hook = lambda h: setattr(mod, "_hook", h)
        antenv.axon_hooks = mod
        sys.modules["antenv.axon_hooks"] = mod
    except Exception:
        pass


def _build_program(W3P, W3, NROWP):
    """Per-core Bass program (same on all 8 cores): outT = (M^T @ W)^T.

    dram "mw" = [W (W3P cols) | M (NROWP cols)] bf16. Matmul (c, m):
    stationary = W cols [128c, 128c+cw), moving = M chunk m -> psum bf16
    [cw, mlen], copied to a staging tile and DMA'd per column chunk.
    """
    import concourse.bacc as bacc
    import concourse.tile as tile
    import concourse.mybir as mybir

    BF16 = mybir.dt.bfloat16
    F32 = mybir.dt.float32

    CW0, CW1 = 128, W3 - 128          # column-chunk widths (128 + 61)
    MC = 512                          # moving chunk (psum bank = 512 fp32)
    NMC = (NROWP + MC - 1) // MC

    nc = bacc.Bacc("TRN2", target_bir_lowering=False, debug=False)
    mw_h = nc.dram_tensor("mw", [R, W3P + NROWP], BF16, kind="ExternalInput")
    # both column chunks padded to 128 partitions: a <128-partition DMA gets
    # all its descriptors packed onto ONE SDMA engine (26 GB/s); 128-partition
    # transfers spread over all 16.
    o_h = nc.dram_tensor("o", [128, 2 * NROWP], BF16, kind="ExternalOutput")

    with tile.TileContext(nc) as tc:
        with (
            tc.tile_pool(name="const", bufs=1) as cpool,
            tc.tile_pool(name="ps", bufs=4, space="PSUM") as hpool,
        ):
            mw = cpool.tile([R, W3P + NROWP], BF16)
            # GpSimd (SWDGE) issues the input DMA: it exits the engine
            # prologue ~2us before Sync/Scalar, hiding the HBM latency
            nc.gpsimd.dma_start(out=mw[:, :], in_=mw_h[:, :])
            st = cpool.tile([128, 2 * NROWP], BF16)
            cut = NROWP + (NMC - 1) * MC
            for ci, (c0, cw) in enumerate([(0, CW0), (CW0, CW1)]):
                for mi in range(NMC):
                    ma = W3P + mi * MC
                    mb = min(W3P + NROWP, ma + MC)
                    ps = hpool.tile([128, MC], F32, tag="ps")
                    nc.tensor.matmul(ps[:cw, :mb - ma], mw[:, c0:c0 + cw],
                                     mw[:, ma:mb], start=True, stop=True)
                    # split each PSUM->SBUF copy across DVE + ACT to halve lag
                    w = mb - ma
                    h = w // 2
                    dst0 = ci * NROWP + ma - W3P
                    nc.vector.tensor_copy(st[:cw, dst0:dst0 + h],
                                          ps[:cw, :h])
                    nc.scalar.copy(st[:cw, dst0 + h:dst0 + w],
                                   ps[:cw, h:w])
                    # out DMAs, earliest-gating order on the sync ring so
                    # the final (scalar-ring) one covers only the last chunk
                    if ci == 0 and mi == NMC - 1:
                        nc.sync.dma_start(out=o_h[:, :NROWP],
                                          in_=st[:, :NROWP])
                    elif ci == 1 and mi == NMC - 2:
                        nc.sync.dma_start(out=o_h[:, NROWP:cut],
                                          in_=st[:, NROWP:cut])
                    elif ci == 1 and mi == NMC - 1:
                        nc.scalar.dma_start(out=o_h[:, cut:2 * NROWP],
                                            in_=st[:, cut:2 * NROWP])
    nc.compile()
    return nc


def _cheb_factors(d1, d2, c, r):
    """Rank-r separation gg(d1_i - d2_j) ~= Phi[:, i]^T Psi[:, j]."""
    lo = min(d1.min(), d2.min())
    hi = max(d1.max(), d2.max())
    mid = 0.5 * (lo + hi)
    half = 0.5 * (hi - lo) * 1.0000001 + 1e-12

    K = CHEB_K
    m = np.arange(K)
    xg = np.cos(np.pi * (m + 0.5) / K)  # chebyshev roots grid

    def gg_fn(diff):
        e = np.exp(-0.5 * c * diff * diff)
        return (c - diff * diff * c * c) * e

    F = gg_fn(half * (xg[:, None] - xg[None, :]))
    T = np.cos(np.pi * np.outer(m + 0.5, m) / K)  # T[m, p] = T_p(x_m)
    C = (2.0 / K) ** 2 * (T.T @ F @ T)
    C[0, :] *= 0.5
    C[:, 0] *= 0.5
    U, S, Vt = np.linalg.svd(C)
    r = int(min(r, K))
    cu = U[:, :r] * np.sqrt(S[:r])
    cv = Vt[:r].T * np.sqrt(S[:r])
    Phi = np.polynomial.chebyshev.chebval((d1 - mid) / half, cu)  # [r, n1]
    Psi = np.polynomial.chebyshev.chebval((d2 - mid) / half, cv)  # [r, n2]
    return Phi, Psi


def kernel(**inputs):
    global LAST_RESULTS
    import ml_dtypes

    d1 = np.asarray(inputs["d1"], dtype=np.float64).reshape(-1)
    u1 = np.asarray(inputs["u1"], dtype=np.float64)
    d2 = np.asarray(inputs["d2"], dtype=np.float64).reshape(-1)
    u2 = np.asarray(inputs["u2"], dtype=np.float64)
    ls = float(np.asarray(inputs["lengthscale"]).reshape(-1)[0])
    i1 = np.asarray(inputs["i1"]).reshape(-1).astype(np.int64)
    i2 = np.asarray(inputs["i2"]).reshape(-1).astype(np.int64)
    na1 = int(np.asarray(inputs["natoms1"]))
    na2 = int(np.asarray(inputs["natoms2"]))
    c = 1.0 / (ls * ls)

    Phi, Psi = _cheb_factors(d1, d2, c, R)

    # fold u1 + segment-sum over i1 into the shared row factor M [R, 3*na1]
    NROW = 3 * na1
    NROWP = (NROW + 127) // 128 * 128
    Mt = np.zeros((NROWP, R))
    for a in range(3):
        np.add.at(Mt, 3 * i1 + a, (Phi * u1[:, a]).T)
    M = Mt.T  # [R, NROWP]

    # fold u2 + segment-sum over i2 into the column factor W [R, na2, 3]
    Wt = np.zeros((3 * na2, R))
    for b in range(3):
        np.add.at(Wt, 3 * i2 + b, (Psi * u2[:, b]).T)
    W_full = Wt.T.reshape(R, na2, 3)

    # column strips: first (na2 % NCORES) cores get one extra atom
    base = na2 // NCORES
    rem = na2 % NCORES
    counts = [base + (1 if cc < rem else 0) for cc in range(NCORES)]
    starts = np.concatenate([[0], np.cumsum(counts)]).astype(np.int64)
    wmax = max(counts)
    W3 = 3 * wmax
    W3P = ((W3 + 127) // 128) * 128

    key = (W3P, W3, NROWP)
    nc = _PROGRAM_CACHE.get(key)
    if nc is None:
        nc = _build_program(W3P, W3, NROWP)
        _PROGRAM_CACHE[key] = nc

    bf16 = ml_dtypes.bfloat16
    in_maps = []
    for cc in range(NCORES):
        a0, a1 = int(starts[cc]), int(starts[cc + 1])
        mw = np.zeros((R, W3P + NROWP), np.float32)
        mw[:, :3 * (a1 - a0)] = W_full[:, a0:a1, :].reshape(R, -1)
        mw[:, W3P:] = M
        in_maps.append({"mw": mw.astype(bf16)})

    from concourse.bass_utils import run_bass_kernel_spmd
    if TRACE:
        _install_ntff_hook()
    res = run_bass_kernel_spmd(nc, in_maps, core_ids=list(range(NCORES)),
                               trace=TRACE)
    LAST_RESULTS = res

    out = np.zeros((3 * na1, 3 * na2), np.float32)
    for cc in range(NCORES):
        a0, a1 = int(starts[cc]), int(starts[cc + 1])
        w3 = 3 * (a1 - a0)
        o = np.asarray(res.results[cc]["o"], dtype=np.float32)
        oT = np.concatenate([o[:, :NROWP], o[:, NROWP:]], axis=0)
        out[:, 3 * a0:3 * a0 + w3] = oT[:w3, :NROW].T
    return out


# revision 9
# speedup vs baseline: 1.2376x; 1.2376x over previous
"""DSimilarity.gradgrad force-force covariance block on 8 Trainium2 cores.

out[m*3+a, n*3+b] = sum_{i,j} u1[i,a]*u2[j,b]*gg[i,j]*[i1[i]==m]*[i2[j]==n]
with gg[i,j] = (c - c^2 diff^2) * exp(-0.5 c diff^2), diff = d1[i]-d2[j], c=1/l^2.

gg depends only on the scalar difference d1[i]-d2[j], so the 4000x4000 kernel
matrix separates: a 2D Chebyshev expansion of gg on the observed d-range,
truncated by SVD, gives gg ~= sum_k phi_k(d1) psi_k(d2) with rank ~16 at
machine precision (rank 32 used for margin). Folding the scatter matrices in
on the host reduces the whole computation to out = M^T @ W with
M[k, 3m+a] = sum_{i1[i]=m} phi_k(d1_i) u1[i,a]   (shared by all cores)
W[k, 3n+b] = sum_{i2[j]=n} psi_k(d2_j) u2[j,b]   (column strip per core).

Each core computes a 62-63-atom strip of output columns, transposed
(out columns on partitions, rows on the free axis) so the whole strip is
4 bf16 matmuls: 2 stationary W-chunks x 2 moving M-chunks (1024+512).
All IO is bf16 (~0.7MB/core); out rides both HWDGE rings.
"""

import math
import sys
import types

import numpy as np

NCORES = 8
CHEB_K = 64  # chebyshev grid size for the 2D expansion
R = 32       # separation rank (machine precision by ~24)

TRACE = False  # test.py sets True to capture an NTFF profile
LAST_RESULTS = None  # BassKernelResults of the last run (for test.py)

_PROGRAM_CACHE = {}


def _install_ntff_hook():
    try:
        from antenv.axon_hooks import get_axon_ntff_profile_hook  # noqa: F401
        return
    except ImportError:
        pass
    try:
        from trn_agent_boot.trn_boot import _ntff_profile_via_ctypes
        import antenv
        hook = _ntff_profile_via_ctypes('/opt/axon/libaxon_pjrt.so')
        mod = types.ModuleType("antenv.axon_hooks")
        mod._hook = hook
        mod.get_axon_ntff_profile_hook = lambda: mod._hook
        mod.set_axon_ntff_profile_hook = lambda h: setattr(mod, "_hook", h)
        antenv.axon_hooks = mod
        sys.modules["antenv.axon_hooks"] = mod
    except Exception:
        pass


def _build_program(W3P, W3, NROWP):
    """Per-core Bass program (same on all 8 cores): outT = (M^T @ W)^T.

    dram "mw" = [W (W3P cols) | M (NROWP cols)] bf16. Matmul (c, m):
    stationary = W cols [128c, 128c+cw), moving = M chunk m -> psum bf16
    [cw, mlen], copied to a staging tile and DMA'd per column chunk.
    """
    import concourse.bacc as bacc
    import concourse.tile as tile
    import concourse.mybir as mybir

    BF16 = mybir.dt.bfloat16
    F32 = mybir.dt.float32

    CW0, CW1 = 128, W3 - 128          # column-chunk widths (128 + 61)
    MC = 512                          # moving chunk (psum bank = 512 fp32)
    NMC = (NROWP + MC - 1) // MC

    nc = bacc.Bacc("TRN2", target_bir_lowering=False, debug=False)
    mw_h = nc.dram_tensor("mw", [R, W3P + NROWP], BF16, kind="ExternalInput")
    # both column chunks padded to 128 partitions: a <128-partition DMA gets
    # all its descriptors packed onto ONE SDMA engine (26 GB/s); 128-partition
    # transfers spread over all 16.
    o_h = nc.dram_tensor("o", [128, 2 * NROWP], BF16, kind="ExternalOutput")

    with tile.TileContext(nc) as tc:
        with (
            tc.tile_pool(name="const", bufs=1) as cpool,
            tc.tile_pool(name="ps", bufs=6, space="PSUM") as hpool,
        ):
            # dummy activation: pulls ACT_TABLE_LOAD into the input-DMA wait
            # (otherwise it lands right before the first PSUM copy and stalls
            # the whole copy chain behind its 1.5us table fetch)
            warm = cpool.tile([1, 8], F32)
            nc.vector.memset(warm[:, :], 0.0)
            nc.scalar.activation(warm[:, :], warm[:, :],
                                 mybir.ActivationFunctionType.Square)
            mw = cpool.tile([R, W3P + NROWP], BF16)
            # GpSimd (SWDGE) issues the head DMA (W + first moving chunk):
            # it clears the init barrier first, so this is the earliest
            # possible issue point; the tail rides the Sync HWDGE ring
            head = W3P + MC
            nc.gpsimd.dma_start(out=mw[:, :head], in_=mw_h[:, :head])
            nc.sync.dma_start(out=mw[:, head:], in_=mw_h[:, head:])
            st = cpool.tile([128, 2 * NROWP], BF16)
            cut = NROWP + (NMC - 1) * MC
            for ci, (c0, cw) in enumerate([(0, CW0), (CW0, CW1)]):
                for mi in range(NMC):
                    ma = W3P + mi * MC
                    mb = min(W3P + NROWP, ma + MC)
                    ps = hpool.tile([128, MC], F32, tag="ps")
                    nc.tensor.matmul(ps[:cw, :mb - ma], mw[:, c0:c0 + cw],
                                     mw[:, ma:mb], start=True, stop=True)
                    # split each PSUM->SBUF copy across DVE + ACT to halve lag
                    w = mb - ma
                    h = w // 2
                    dst0 = ci * NROWP + ma - W3P
                    nc.vector.tensor_copy(st[:cw, dst0:dst0 + h],
                                          ps[:cw, :h])
                    nc.scalar.copy(st[:cw, dst0 + h:dst0 + w],
                                   ps[:cw, h:w])
                    # out DMAs, earliest-gating order on the sync ring so
                    # the final (scalar-ring) one covers only the last chunk
                    if ci == 0 and mi == NMC - 1:
                        nc.sync.dma_start(out=o_h[:, :NROWP],
                                          in_=st[:, :NROWP])
                    elif ci == 1 and mi == NMC - 2:
                        nc.sync.dma_start(out=o_h[:, NROWP:cut],
                                          in_=st[:, NROWP:cut])
                    elif ci == 1 and mi == NMC - 1:
                        nc.scalar.dma_start(out=o_h[:, cut:2 * NROWP],
                                            in_=st[:, cut:2 * NROWP])
    nc.compile()
    return nc


def _cheb_factors(d1, d2, c, r):
    """Rank-r separation gg(d1_i - d2_j) ~= Phi[:, i]^T Psi[:, j]."""
    lo = min(d1.min(), d2.min())
    hi = max(d1.max(), d2.max())
    mid = 0.5 * (lo + hi)
    half = 0.5 * (hi - lo) * 1.0000001 + 1e-12

    K = CHEB_K
    m = np.arange(K)
    xg = np.cos(np.pi * (m + 0.5) / K)  # chebyshev roots grid

    def gg_fn(diff):
        e = np.exp(-0.5 * c * diff * diff)
        return (c - diff * diff * c * c) * e

    F = gg_fn(half * (xg[:, None] - xg[None, :]))
    T = np.cos(np.pi * np.outer(m + 0.5, m) / K)  # T[m, p] = T_p(x_m)
    C = (2.0 / K) ** 2 * (T.T @ F @ T)
    C[0, :] *= 0.5
    C[:, 0] *= 0.5
    U, S, Vt = np.linalg.svd(C)
    r = int(min(r, K))
    cu = U[:, :r] * np.sqrt(S[:r])
    cv = Vt[:r].T * np.sqrt(S[:r])
    Phi = np.polynomial.chebyshev.chebval((d1 - mid) / half, cu)  # [r, n1]
    Psi = np.polynomial.chebyshev.chebval((d2 - mid) / half, cv)  # [r, n2]
    return Phi, Psi


def kernel(**inputs):
    global LAST_RESULTS
    import ml_dtypes

    d1 = np.asarray(inputs["d1"], dtype=np.float64).reshape(-1)
    u1 = np.asarray(inputs["u1"], dtype=np.float64)
    d2 = np.asarray(inputs["d2"], dtype=np.float64).reshape(-1)
    u2 = np.asarray(inputs["u2"], dtype=np.float64)
    ls = float(np.asarray(inputs["lengthscale"]).reshape(-1)[0])
    i1 = np.asarray(inputs["i1"]).reshape(-1).astype(np.int64)
    i2 = np.asarray(inputs["i2"]).reshape(-1).astype(np.int64)
    na1 = int(np.asarray(inputs["natoms1"]))
    na2 = int(np.asarray(inputs["natoms2"]))
    c = 1.0 / (ls * ls)

    Phi, Psi = _cheb_factors(d1, d2, c, R)

    # fold u1 + segment-sum over i1 into the shared row factor M [R, 3*na1]
    NROW = 3 * na1
    NROWP = (NROW + 127) // 128 * 128
    Mt = np.zeros((NROWP, R))
    for a in range(3):
        np.add.at(Mt, 3 * i1 + a, (Phi * u1[:, a]).T)
    M = Mt.T  # [R, NROWP]

    # fold u2 + segment-sum over i2 into the column factor W [R, na2, 3]
    Wt = np.zeros((3 * na2, R))
    for b in range(3):
        np.add.at(Wt, 3 * i2 + b, (Psi * u2[:, b]).T)
    W_full = Wt.T.reshape(R, na2, 3)

    # column strips: first (na2 % NCORES) cores get one extra atom
    base = na2 // NCORES
    rem = na2 % NCORES
    counts = [base + (1 if cc < rem else 0) for cc in range(NCORES)]
    starts = np.concatenate([[0], np.cumsum(counts)]).astype(np.int64)
    wmax = max(counts)
    W3 = 3 * wmax
    W3P = ((W3 + 127) // 128) * 128

    key = (W3P, W3, NROWP)
    nc = _PROGRAM_CACHE.get(key)
    if nc is None:
        nc = _build_program(W3P, W3, NROWP)
        _PROGRAM_CACHE[key] = nc

    bf16 = ml_dtypes.bfloat16
    in_maps = []
    for cc in range(NCORES):
        a0, a1 = int(starts[cc]), int(starts[cc + 1])
        mw = np.zeros((R, W3P + NROWP), np.float32)
        mw[:, :3 * (a1 - a0)] = W_full[:, a0:a1, :].reshape(R, -1)
        mw[:, W3P:] = M
        in_maps.append({"mw": mw.astype(bf16)})

    from concourse.bass_utils import run_bass_kernel_spmd
    if TRACE:
        _install_ntff_hook()
    res = run_bass_kernel_spmd(nc, in_maps, core_ids=list(range(NCORES)),
                               trace=TRACE)
    LAST_RESULTS = res

    out = np.zeros((3 * na1, 3 * na2), np.float32)
    for cc in range(NCORES):
        a0, a1 = int(starts[cc]), int(starts[cc + 1])
        w3 = 3 * (a1 - a0)
        o = np.asarray(res.results[cc]["o"], dtype=np.float32)
        oT = np.concatenate([o[:, :NROWP], o[:, NROWP:]], axis=0)
        out[:, 3 * a0:3 * a0 + w3] = oT[:w3, :NROW].T
    return out


# revision 11
# speedup vs baseline: 1.2643x; 1.0216x over previous
"""DSimilarity.gradgrad force-force covariance block on 8 Trainium2 cores.

out[m*3+a, n*3+b] = sum_{i,j} u1[i,a]*u2[j,b]*gg[i,j]*[i1[i]==m]*[i2[j]==n]
with gg[i,j] = (c - c^2 diff^2) * exp(-0.5 c diff^2), diff = d1[i]-d2[j], c=1/l^2.

gg depends only on the scalar difference d1[i]-d2[j], so the 4000x4000 kernel
matrix separates: a 2D Chebyshev expansion of gg on the observed d-range,
truncated by SVD, gives gg ~= sum_k phi_k(d1) psi_k(d2) with rank ~16 at
machine precision (rank 32 used for margin). Folding the scatter matrices in
on the host reduces the whole computation to out = M^T @ W with
M[k, 3m+a] = sum_{i1[i]=m} phi_k(d1_i) u1[i,a]   (shared by all cores)
W[k, 3n+b] = sum_{i2[j]=n} psi_k(d2_j) u2[j,b]   (column strip per core).

Each core computes a 62-63-atom strip of output columns, transposed
(out columns on partitions, rows on the free axis) so the whole strip is
4 bf16 matmuls: 2 stationary W-chunks x 2 moving M-chunks (1024+512).
All IO is bf16 (~0.7MB/core); out rides both HWDGE rings.
"""

import math
import sys
import types

import numpy as np

NCORES = 8
CHEB_K = 64  # chebyshev grid size for the 2D expansion
R = 32       # separation rank (machine precision by ~24)

TRACE = False  # test.py sets True to capture an NTFF profile
LAST_RESULTS = None  # BassKernelResults of the last run (for test.py)

_PROGRAM_CACHE = {}


def _install_ntff_hook():
    try:
        from antenv.axon_hooks import get_axon_ntff_profile_hook  # noqa: F401
        return
    except ImportError:
        pass
    try:
        from trn_agent_boot.trn_boot import _ntff_profile_via_ctypes
        import antenv
        hook = _ntff_profile_via_ctypes('/opt/axon/libaxon_pjrt.so')
        mod = types.ModuleType("antenv.axon_hooks")
        mod._hook = hook
        mod.get_axon_ntff_profile_hook = lambda: mod._hook
        mod.set_axon_ntff_profile_hook = lambda h: setattr(mod, "_hook", h)
        antenv.axon_hooks = mod
        sys.modules["antenv.axon_hooks"] = mod
    except Exception:
        pass


def _build_program(W3P, W3, NROWP):
    """Per-core Bass program (same on all 8 cores): outT = (M^T @ W)^T.

    dram "mw" = [W (W3P cols) | M (NROWP cols)] bf16. Matmul (c, m):
    stationary = W cols [128c, 128c+cw), moving = M chunk m -> psum bf16
    [cw, mlen], copied to a staging tile and DMA'd per column chunk.
    """
    import concourse.bacc as bacc
    import concourse.tile as tile
    import concourse.mybir as mybir

    BF16 = mybir.dt.bfloat16
    F32 = mybir.dt.float32

    CW0, CW1 = 128, W3 - 128          # column-chunk widths (128 + 61)
    # moving chunks (psum bank max = 512 fp32); tail chunks smaller so the
    # last copy + last out-DMA piece gating the epilogue are short
    MCS = [512] * (NROWP // 512 - 1) + [256, 256] if NROWP >= 1024 \
        else [512] * ((NROWP + 511) // 512)
    assert sum(MCS) == NROWP

    nc = bacc.Bacc("TRN2", target_bir_lowering=False, debug=False)
    mw_h = nc.dram_tensor("mw", [R, W3P + NROWP], BF16, kind="ExternalInput")
    # both column chunks padded to 128 partitions: a <128-partition DMA gets
    # all its descriptors packed onto ONE SDMA engine (26 GB/s); 128-partition
    # transfers spread over all 16.
    o_h = nc.dram_tensor("o", [128, 2 * NROWP], BF16, kind="ExternalOutput")

    with tile.TileContext(nc) as tc:
        with (
            tc.tile_pool(name="const", bufs=1) as cpool,
            tc.tile_pool(name="ps5", bufs=4, space="PSUM") as hpool5,
            tc.tile_pool(name="ps2", bufs=3, space="PSUM") as hpool2,
        ):
            # dummy activation: pulls ACT_TABLE_LOAD into the input-DMA wait
            # (otherwise it lands right before the first PSUM copy and stalls
            # the whole copy chain behind its 1.5us table fetch)
            warm = cpool.tile([1, 8], F32)
            nc.vector.memset(warm[:, :], 0.0)
            nc.scalar.activation(warm[:, :], warm[:, :],
                                 mybir.ActivationFunctionType.Square)
            mw = cpool.tile([R, W3P + NROWP], BF16)
            # Sync clears the engine prologue first -> earliest issue point
            nc.sync.dma_start(out=mw[:, :], in_=mw_h[:, :])
            st = cpool.tile([128, 2 * NROWP], BF16)
            NMC = len(MCS)
            cut = NROWP + sum(MCS[:-1])
            for ci, (c0, cw) in enumerate([(0, CW0), (CW0, CW1)]):
                ma = W3P
                for mi, mc in enumerate(MCS):
                    mb = ma + mc
                    last = ci == 1 and mi == NMC - 1
                    if mc > 256:
                        ps = hpool5.tile([128, 512], F32, tag="p5")
                    else:
                        ps = hpool2.tile([128, 256], F32, tag="p2")
                    nc.tensor.matmul(ps[:cw, :mc], mw[:, c0:c0 + cw],
                                     mw[:, ma:mb], start=True, stop=True)
                    dst0 = ci * NROWP + ma - W3P
                    if last:
                        # single fast DVE copy so only one engine gates the
                        # final out-DMA piece
                        nc.vector.tensor_copy(st[:cw, dst0:dst0 + mc],
                                              ps[:cw, :mc])
                    else:
                        h = mc // 2
                        nc.vector.tensor_copy(st[:cw, dst0:dst0 + h],
                                              ps[:cw, :h])
                        nc.scalar.copy(st[:cw, dst0 + h:dst0 + mc],
                                       ps[:cw, h:mc])
                    # out DMAs, earliest-gating order on the sync ring so
                    # the final (scalar-ring) one covers only the last chunk
                    if ci == 0 and mi == NMC - 1:
                        nc.sync.dma_start(out=o_h[:, :NROWP],
                                          in_=st[:, :NROWP])
                    elif ci == 1 and mi == NMC - 2:
                        nc.sync.dma_start(out=o_h[:, NROWP:cut],
                                          in_=st[:, NROWP:cut])
                    elif last:
                        nc.scalar.dma_start(out=o_h[:, cut:2 * NROWP],
                                            in_=st[:, cut:2 * NROWP])
                    ma = mb
    nc.compile()
    return nc


def _cheb_factors(d1, d2, c, r):
    """Rank-r separation gg(d1_i - d2_j) ~= Phi[:, i]^T Psi[:, j]."""
    lo = min(d1.min(), d2.min())
    hi = max(d1.max(), d2.max())
    mid = 0.5 * (lo + hi)
    half = 0.5 * (hi - lo) * 1.0000001 + 1e-12

    K = CHEB_K
    m = np.arange(K)
    xg = np.cos(np.pi * (m + 0.5) / K)  # chebyshev roots grid

    def gg_fn(diff):
        e = np.exp(-0.5 * c * diff * diff)
        return (c - diff * diff * c * c) * e

    F = gg_fn(half * (xg[:, None] - xg[None, :]))
    T = np.cos(np.pi * np.outer(m + 0.5, m) / K)  # T[m, p] = T_p(x_m)
    C = (2.0 / K) ** 2 * (T.T @ F @ T)
    C[0, :] *= 0.5
    C[:, 0] *= 0.5
    U, S, Vt = np.linalg.svd(C)
    r = int(min(r, K))
    cu = U[:, :r] * np.sqrt(S[:r])
    cv = Vt[:r].T * np.sqrt(S[:r])
    Phi = np.polynomial.chebyshev.chebval((d1 - mid) / half, cu)  # [r, n1]
    Psi = np.polynomial.chebyshev.chebval((d2 - mid) / half, cv)  # [r, n2]
    return Phi, Psi


def kernel(**inputs):
    global LAST_RESULTS
    import ml_dtypes

    d1 = np.asarray(inputs["d1"], dtype=np.float64).reshape(-1)
    u1 = np.asarray(inputs["u1"], dtype=np.float64)
    d2 = np.asarray(inputs["d2"], dtype=np.float64).reshape(-1)
    u2 = np.asarray(inputs["u2"], dtype=np.float64)
    ls = float(np.asarray(inputs["lengthscale"]).reshape(-1)[0])
    i1 = np.asarray(inputs["i1"]).reshape(-1).astype(np.int64)
    i2 = np.asarray(inputs["i2"]).reshape(-1).astype(np.int64)
    na1 = int(np.asarray(inputs["natoms1"]))
    na2 = int(np.asarray(inputs["natoms2"]))
    c = 1.0 / (ls * ls)

    Phi, Psi = _cheb_factors(d1, d2, c, R)

    # fold u1 + segment-sum over i1 into the shared row factor M [R, 3*na1]
    NROW = 3 * na1
    NROWP = (NROW + 127) // 128 * 128
    Mt = np.zeros((NROWP, R))
    for a in range(3):
        np.add.at(Mt, 3 * i1 + a, (Phi * u1[:, a]).T)
    M = Mt.T  # [R, NROWP]

    # fold u2 + segment-sum over i2 into the column factor W [R, na2, 3]
    Wt = np.zeros((3 * na2, R))
    for b in range(3):
        np.add.at(Wt, 3 * i2 + b, (Psi * u2[:, b]).T)
    W_full = Wt.T.reshape(R, na2, 3)

    # column strips: first (na2 % NCORES) cores get one extra atom
    base = na2 // NCORES
    rem = na2 % NCORES
    counts = [base + (1 if cc < rem else 0) for cc in range(NCORES)]
    starts = np.concatenate([[0], np.cumsum(counts)]).astype(np.int64)
    wmax = max(counts)
    W3 = 3 * wmax
    W3P = ((W3 + 127) // 128) * 128

    key = (W3P, W3, NROWP)
    nc = _PROGRAM_CACHE.get(key)
    if nc is None:
        nc = _build_program(W3P, W3, NROWP)
        _PROGRAM_CACHE[key] = nc

    bf16 = ml_dtypes.bfloat16
    in_maps = []
    for cc in range(NCORES):
        a0, a1 = int(starts[cc]), int(starts[cc + 1])
        mw = np.zeros((R, W3P + NROWP), np.float32)
        mw[:, :3 * (a1 - a0)] = W_full[:, a0:a1, :].reshape(R, -1)
        mw[:, W3P:] = M
        in_maps.append({"mw": mw.astype(bf16)})

    from concourse.bass_utils import run_bass_kernel_spmd
    if TRACE:
        _install_ntff_hook()
    res = run_bass_kernel_spmd(nc, in_maps, core_ids=list(range(NCORES)),
                               trace=TRACE)
    LAST_RESULTS = res

    out = np.zeros((3 * na1, 3 * na2), np.float32)
    for cc in range(NCORES):
        a0, a1 = int(starts[cc]), int(starts[cc + 1])
        w3 = 3 * (a1 - a0)
        o = np.asarray(res.results[cc]["o"], dtype=np.float32)
        oT = np.concatenate([o[:, :NROWP], o[:, NROWP:]], axis=0)
        out[:, 3 * a0:3 * a0 + w3] = oT[:w3, :NROW].T
    return out
